# revision 14
# baseline (speedup 1.0000x reference)
"""Trainium2 Bass kernel for batched self-attention with a linear projection.

Reference (per batch element b):
    proj  = x @ W.T + bias              # [S, D]
    S_ij  = x_i . proj_j                # [S, S]
    A     = softmax(S, axis=-1)         # [S, S]
    ctx   = A @ x                       # [S, D]
returns (ctx, A).

Sharding: data-parallel over batch B=8 across the 8 NeuronCores (one batch
element per core); W and bias replicated. No collectives needed.

Per-core layout (P = 128 partitions):
  xT[d, j]     = x[j, d]      (for contractions over the feature dim)
  wt[d, f]     = W[f, d]
  projF[f, j]  = proj[j, f]
  scores tile: S[i, j] with i on partitions, softmax along free axis j,
  then A is PE-transposed per 128x128 block to feed the context matmul
  (lhsT must have the contraction dim j on partitions).

The context matmul and the A-transpose run in bf16 (A in [0,1], x ~ N(0,1);
relative error ~2^-9, far below the harness gate), while proj/scores run in
fp32 (softmax amplifies score error exp()-fold, scores have std ~sqrt(D)).
"""

import os
import numpy as np

import concourse.bass as bass
import concourse.mybir as mybir
import concourse.tile as tile
from concourse import bacc
from concourse.bass_utils import run_bass_kernel_spmd
from concourse.masks import make_identity

P = 128
S = 2048
D = 1024
NIT = S // P   # 16 row (i / j) tiles
NDT = D // P   # 8 feature (d / f) tiles
NJC = S // 512  # 4 free-dim chunks of 512 for the scores matmul
NEC = D // 512  # 2 free-dim chunks for proj/context
N_CORES = 8

F32 = mybir.dt.float32
F32R = mybir.dt.float32r
BF16 = mybir.dt.bfloat16

# fp32r streams at 1 cycle/row (vs fp32's 4) for moving dim >= 256, at ~14
# effective mantissa bits (hardware-measured: rms err 2.9e-3 on a K=1024
# N(0,1) matmul, i.e. ~0.3% softmax weight error here — well under the
# accuracy gate). fp32r operands must be produced by a rounding instruction
# (DVE copy with float32r output) or DMA'd from an f32r-typed buffer.
USE_F32R = bool(int(os.environ.get("KERNEL_F32R", "1")))


def build(nc):
    x_ext = nc.dram_tensor("x", [S, D], F32, kind="ExternalInput").ap()
    w_ext = nc.dram_tensor("w", [D, D], F32, kind="ExternalInput").ap()
    b_ext = nc.dram_tensor("bias", [D], F32, kind="ExternalInput").ap()
    ctx_ext = nc.dram_tensor("ctx", [S, D], F32, kind="ExternalOutput").ap()
    attn_ext = nc.dram_tensor("attn", [S, S], F32, kind="ExternalOutput").ap()

    sdt = F32R if USE_F32R else F32
    xT_dram = nc.dram_tensor("xT_stage", [D, S], sdt).ap()

    with tile.TileContext(nc) as tc:
        with tc.tile_pool(name="big", bufs=1) as big:
            # Persistent SBUF residents.
            projF = big.tile([P, NDT, S], sdt)     # 64 KiB/partition
            xbf = big.tile([P, NIT, D], BF16)      # 32 KiB/partition
            ident32 = big.tile([P, P], F32)
            ident16 = big.tile([P, P], BF16)
            b_sb = big.tile([P, NDT], F32)
            make_identity(nc, ident32[:])
            make_identity(nc, ident16[:])
            nc.sync.dma_start(b_sb[:], b_ext.rearrange("(ft p) -> p ft", p=P))

            # ---- Phase 0: load x/W, cast x to bf16, transpose both on PE.
            # ---- Phase 1: projF[f, j] = sum_d wt[d, f] * xT[d, j] + b[f].
            with (
                tc.tile_pool(name="ph01", bufs=3) as ph01,
                tc.tile_pool(name="wtp", bufs=1) as wtp,
                tc.tile_pool(name="ps0", bufs=4, space=bass.MemorySpace.PSUM) as ps0,
            ):
                wt = wtp.tile([P, NDT, D], sdt)    # 32 KiB/partition, phase 0/1 only

                for ft in range(NDT):
                    wrow = ph01.tile([P, D], F32, tag="row")
                    nc.sync.dma_start(wrow[:], w_ext[ft * P:(ft + 1) * P, :])
                    for dt in range(NDT):
                        trp = ps0.tile([P, P], F32, tag="tr")
                        nc.tensor.transpose(trp[:], wrow[:, dt * P:(dt + 1) * P], ident32[:])
                        nc.vector.tensor_copy(wt[:, dt, ft * P:(ft + 1) * P], trp[:])

                for jt in range(NIT):
                    xrow = ph01.tile([P, D], F32, tag="row")
                    nc.sync.dma_start(xrow[:], x_ext[jt * P:(jt + 1) * P, :])
                    nc.vector.tensor_copy(xbf[:, jt, :], xrow[:])
                    for dt in range(NDT):
                        trp = ps0.tile([P, P], F32, tag="tr")
                        nc.tensor.transpose(trp[:], xrow[:, dt * P:(dt + 1) * P], ident32[:])
                        xts = ph01.tile([P, P], sdt, tag="xts")
                        nc.vector.tensor_copy(xts[:], trp[:])
                        nc.sync.dma_start(
                            xT_dram[dt * P:(dt + 1) * P, jt * P:(jt + 1) * P], xts[:]
                        )

                xT_view = xT_dram.rearrange("(dt p) j -> p dt j", p=P)
                for jc in range(NJC):
                    xTjc = ph01.tile([P, NDT, 512], sdt, tag="xTjc", bufs=2)
                    nc.sync.dma_start(xTjc[:], xT_view[:, :, jc * 512:(jc + 1) * 512])
                    for ft in range(NDT):
                        pp = ps0.tile([P, 512], F32, tag="pp")
                        for dt in range(NDT):
                            nc.tensor.matmul(
                                pp[:],
                                wt[:, dt, ft * P:(ft + 1) * P],
                                xTjc[:, dt, :],
                                start=(dt == 0),
                                stop=(dt == NDT - 1),
                            )
                        nc.vector.tensor_scalar_add(
                            projF[:, ft, jc * 512:(jc + 1) * 512], pp[:], b_sb[:, ft:ft + 1]
                        )

            # ---- Phase 2: per 128-row tile: scores -> softmax -> A out;
            # transpose A (bf16) and context matmul, software-pipelined one
            # iteration behind so the PE never waits on the softmax chain.
            with (
                tc.tile_pool(name="work", bufs=1) as work,
                tc.tile_pool(name="ps_s", bufs=1, space=bass.MemorySpace.PSUM) as ps_s,
                tc.tile_pool(name="ps_b", bufs=2, space=bass.MemorySpace.PSUM) as ps_b,
            ):
                a16_tiles = [None] * NIT

                def stage_a(it):
                    xTi = work.tile([P, NDT, P], sdt, tag="xTi", bufs=2)
                    nc.sync.dma_start(xTi[:], xT_view[:, :, it * P:(it + 1) * P])
                    sps = ps_s.tile([P, S], F32, tag="sps")
                    # ft outer / jc inner: the stationary operand xTi[:, ft]
                    # is loaded once per ft and reused across the 4 free-dim
                    # chunks; each chunk is its own PSUM accumulation group.
                    for ft in range(NDT):
                        for jc in range(NJC):
                            nc.tensor.matmul(
                                sps[:, jc * 512:(jc + 1) * 512],
                                xTi[:, ft, :],
                                projF[:, ft, jc * 512:(jc + 1) * 512],
                                start=(ft == 0),
                                stop=(ft == NDT - 1),
                            )
                    negm = work.tile([P, 1], F32, tag="negm", bufs=2)
                    nc.vector.reduce_max(negm[:], sps[:], axis=mybir.AxisListType.X, negate=True)
                    psb = work.tile([P, S], F32, tag="psb", bufs=2)
                    lsum = work.tile([P, 1], F32, tag="lsum", bufs=2)
                    nc.scalar.activation(
                        psb[:], sps[:], mybir.ActivationFunctionType.Exp,
                        bias=negm[:], accum_out=lsum[:],
                    )
                    rcp = work.tile([P, 1], F32, tag="rcp", bufs=2)
                    nc.vector.reciprocal(rcp[:], lsum[:])
                    nc.vector.tensor_scalar_mul(psb[:], psb[:], rcp[:])
                    nc.sync.dma_start(attn_ext[it * P:(it + 1) * P, :], psb[:])
                    a16 = work.tile([P, S], BF16, tag="a16", bufs=2)
                    nc.scalar.copy(a16[:], psb[:])
                    a16_tiles[it] = a16

                def stage_b(it):
                    a16 = a16_tiles[it]
                    pt = work.tile([P, S], BF16, tag="pt", bufs=2)
                    for q in range(NIT // 4):
                        trp = ps_b.tile([P, 512], BF16, tag="trb")
                        for k in range(4):
                            jt = q * 4 + k
                            nc.tensor.transpose(
                                trp[:, k * P:(k + 1) * P],
                                a16[:, jt * P:(jt + 1) * P],
                                ident16[:],
                            )
                        nc.vector.tensor_copy(pt[:, q * 512:(q + 1) * 512], trp[:])
                    cps = ps_b.tile([P, D], F32, tag="cps", bufs=1)
                    # jt outer / ec inner: stationary pt block reused across
                    # both 512-wide output chunks.
                    for jt in range(NIT):
                        for ec in range(NEC):
                            nc.tensor.matmul(
                                cps[:, ec * 512:(ec + 1) * 512],
                                pt[:, jt * P:(jt + 1) * P],
                                xbf[:, jt, ec * 512:(ec + 1) * 512],
                                start=(jt == 0),
                                stop=(jt == NIT - 1),
                            )
                    csb = work.tile([P, D], F32, tag="csb", bufs=2)
                    nc.vector.tensor_copy(csb[:], cps[:])
                    nc.sync.dma_start(ctx_ext[it * P:(it + 1) * P, :], csb[:])

                for it in range(NIT):
                    stage_a(it)
                    if it >= 1:
                        stage_b(it - 1)
                stage_b(NIT - 1)

    nc.compile()
    return nc


_compiled_nc = None


def _get_nc():
    global _compiled_nc
    if _compiled_nc is None:
        nc = bacc.Bacc("TRN2", target_bir_lowering=False, debug=False)
        _compiled_nc = build(nc)
    return _compiled_nc


LAST_EXEC_NS = None
LAST_RESULTS = None


def kernel(lstm_output, W, b):
    global LAST_EXEC_NS, LAST_RESULTS
    nc = _get_nc()
    x = np.ascontiguousarray(np.asarray(lstm_output, dtype=np.float32))
    w = np.ascontiguousarray(np.asarray(W, dtype=np.float32))
    bias = np.ascontiguousarray(np.asarray(b, dtype=np.float32))
    in_maps = [
        {"x": x[c], "w": w, "bias": bias} for c in range(N_CORES)
    ]
    trace = bool(int(os.environ.get("KERNEL_TRACE", "0")))
    res = run_bass_kernel_spmd(nc, in_maps, core_ids=list(range(N_CORES)), trace=trace)
    LAST_EXEC_NS = res.exec_time_ns
    LAST_RESULTS = res
    ctx = np.stack([res.results[c]["ctx"] for c in range(N_CORES)])
    attn = np.stack([res.results[c]["attn"] for c in range(N_CORES)])
    return ctx, attn


# revision 37
# speedup vs baseline: 8.5447x; 8.5447x over previous
"""Trainium2 Bass kernel for batched self-attention with a linear projection.

Reference (per batch element b):
    proj  = x @ W.T + bias              # [S, D]
    S_ij  = x_i . proj_j                # [S, S]
    A     = softmax(S, axis=-1)         # [S, S]
    ctx   = A @ x                       # [S, D]
returns (ctx, A).

Sharding: data-parallel over batch B=8 across the 8 NeuronCores (one batch
element per core); W and bias replicated. No collectives needed.

Per-core layout (P = 128 partitions):
  xT[d, j]     = x[j, d]      (for contractions over the feature dim)
  wt[d, f]     = W[f, d]
  projF[f, j]  = proj[j, f]
  scores tile: S[i, j] with i on partitions, softmax along free axis j,
  then A is PE-transposed per 128x128 block to feed the context matmul
  (lhsT must have the contraction dim j on partitions).

The context matmul and the A-transpose run in bf16 (A in [0,1], x ~ N(0,1);
relative error ~2^-9, far below the harness gate), while proj/scores run in
fp32 (softmax amplifies score error exp()-fold, scores have std ~sqrt(D)).
"""

import os
import numpy as np

import concourse.bass as bass
import concourse.mybir as mybir
import concourse.tile as tile
from concourse import bacc
from concourse.bass_utils import run_bass_kernel_spmd
from concourse.masks import make_identity

P = 128
S = 2048
D = 1024
NIT = S // P   # 16 row (i / j) tiles
NDT = D // P   # 8 feature (d / f) tiles
NJC = S // 512  # 4 free-dim chunks of 512 for the scores matmul
NEC = D // 512  # 2 free-dim chunks for proj/context
N_CORES = 8

F32 = mybir.dt.float32
F32R = mybir.dt.float32r
BF16 = mybir.dt.bfloat16

# fp32r streams at 1 cycle/row (vs fp32's 4) for moving dim >= 256, at ~14
# effective mantissa bits (hardware-measured: rms err 2.9e-3 on a K=1024
# N(0,1) matmul, i.e. ~0.3% softmax weight error here — well under the
# accuracy gate). fp32r operands must be produced by a rounding instruction
# (DVE copy with float32r output) or DMA'd from an f32r-typed buffer.
USE_F32R = bool(int(os.environ.get("KERNEL_F32R", "1")))


def build(nc, reps=1):
    x_ext = nc.dram_tensor("x", [S, D], F32, kind="ExternalInput").ap()
    w_ext = nc.dram_tensor("w", [D, D], F32, kind="ExternalInput").ap()
    b_ext = nc.dram_tensor("bias", [D], F32, kind="ExternalInput").ap()
    ctx_ext = nc.dram_tensor("ctx", [S, D], F32, kind="ExternalOutput").ap()
    attn_ext = nc.dram_tensor("attn", [S, S], F32, kind="ExternalOutput").ap()

    sdt = F32R if USE_F32R else F32
    # One staging tensor per 512-column group: phase 2's per-row-tile
    # stationary reload then only depends on its own group's staging write
    # (DRAM dependencies are tracked per tensor, not per region).
    xT_stage = [nc.dram_tensor(f"xT_stage{jc}", [D, 512], sdt).ap() for jc in range(NJC)]

    with tile.TileContext(nc) as tc:
        for _rep in range(reps):
            _build_body(nc, tc, x_ext, w_ext, b_ext, ctx_ext, attn_ext, xT_stage, sdt)

    nc.compile()
    return nc


def _build_body(nc, tc, x_ext, w_ext, b_ext, ctx_ext, attn_ext, xT_stage, sdt):
    if True:
        with tc.tile_pool(name="big", bufs=1) as big:
            # Persistent SBUF residents.
            projF = big.tile([P, NDT, S], sdt)     # 64 KiB/partition
            xbf = big.tile([P, NIT, D], BF16)      # 32 KiB/partition
            ident32 = big.tile([P, P], F32)
            ident16 = big.tile([P, P], BF16)
            b_sb = big.tile([P, NDT], F32)
            make_identity(nc, ident32[:])
            make_identity(nc, ident16[:])
            nc.sync.dma_start(b_sb[:], b_ext.rearrange("(ft p) -> p ft", p=P))

            # ---- Phase 0+1, fused per 512-column group: load x rows,
            # PE-transpose into an SBUF-assembled xTjc tile (no DRAM
            # round-trip on the critical path), immediately run the proj
            # matmuls for that column group, and stage xTjc to DRAM for
            # phase 2's per-row-tile stationary reloads.
            xT_views = [t.rearrange("(dt p) j -> p dt j", p=P) for t in xT_stage]
            xti_pre = []
            w_view = w_ext.rearrange("(ft p) d -> p ft d", p=P)
            x_view = x_ext.rearrange("(jt p) d -> p jt d", p=P)
            with (
                tc.tile_pool(name="ph01", bufs=2) as ph01,
                tc.tile_pool(name="wtp", bufs=1) as wtp,
                tc.tile_pool(name="ps0", bufs=4, space=bass.MemorySpace.PSUM) as ps0,
            ):
                wt = wtp.tile([P, NDT, D], sdt)   # 32 KiB/partition, phase 0/1 only

                def x_group(jc):
                    """Load 4 x row-tiles, cast to bf16, PE-transpose into a
                    fresh xTjc tile."""
                    xTjc = ph01.tile([P, NDT, 512], sdt, tag="xTjc", name=f"xTjc{jc}")
                    for h in range(2):
                        xrow2 = ph01.tile([P, 2, D], F32, tag="row2", name=f"xr{jc}_{h}")
                        nc.sync.dma_start(
                            xrow2[:], x_view[:, jc * 4 + 2 * h:jc * 4 + 2 * h + 2, :]
                        )
                        for jl in range(2):
                            jt = jc * 4 + 2 * h + jl
                            nc.scalar.copy(xbf[:, jt, :], xrow2[:, jl, :])
                            for dt in range(NDT):
                                trp = ps0.tile([P, P], F32, tag="tr")
                                nc.tensor.transpose(
                                    trp[:], xrow2[:, jl, dt * P:(dt + 1) * P], ident32[:]
                                )
                                nc.vector.tensor_copy(
                                    xTjc[:, dt, (2 * h + jl) * P:(2 * h + jl + 1) * P], trp[:]
                                )
                    return xTjc

                def proj_group(jc, xTjc):
                    nc.sync.dma_start(xT_views[jc][:], xTjc[:])
                    if jc == 0:
                        # Prefetch phase 2's first two stationary tiles now,
                        # ahead of the later staging DMAs in the sync
                        # sequencer's program order.
                        for pre_it in range(2):
                            xTi = big.tile([P, NDT, P], sdt, tag="xTi", bufs=2,
                                           name=f"xTi_pre{pre_it}")
                            nc.sync.dma_start(
                                xTi[:], xT_views[0][:, :, pre_it * P:(pre_it + 1) * P]
                            )
                            xti_pre.append(xTi)
                    for ft in range(NDT):
                        pp = ps0.tile([P, 512], F32, tag="pp")
                        for dt in range(NDT):
                            nc.tensor.matmul(
                                pp[:],
                                wt[:, dt, ft * P:(ft + 1) * P],
                                xTjc[:, dt, :],
                                start=(dt == 0),
                                stop=(dt == NDT - 1),
                            )
                        nc.vector.tensor_scalar_add(
                            projF[:, ft, jc * 512:(jc + 1) * 512], pp[:], b_sb[:, ft:ft + 1]
                        )

                # x group 0 first: its DMAs lead the sync-sequencer queue and
                # its transposes give the PE early work.
                xTjc0 = x_group(0)
                for wg in range(4):
                    wrow2 = ph01.tile([P, 2, D], F32, tag="row2", name=f"wr{wg}")
                    nc.sync.dma_start(wrow2[:], w_view[:, wg * 2:(wg + 1) * 2, :])
                    for fl in range(2):
                        ft = wg * 2 + fl
                        for dt in range(NDT):
                            trp = ps0.tile([P, P], F32, tag="tr")
                            nc.tensor.transpose(trp[:], wrow2[:, fl, dt * P:(dt + 1) * P], ident32[:])
                            nc.vector.tensor_copy(wt[:, dt, ft * P:(ft + 1) * P], trp[:])
                proj_group(0, xTjc0)
                for jc in range(1, NJC):
                    proj_group(jc, x_group(jc))

            # ---- Phase 2: per 128-row tile: scores -> softmax -> A out;
            # transpose A (bf16) and context matmul, software-pipelined one
            # iteration behind so the PE never waits on the softmax chain.
            with (
                tc.tile_pool(name="work", bufs=1) as work,
                tc.tile_pool(name="ps_s", bufs=1, space=bass.MemorySpace.PSUM) as ps_s,
                tc.tile_pool(name="ps_b", bufs=2, space=bass.MemorySpace.PSUM) as ps_b,
            ):
                a16_tiles = [None] * NIT

                def stage_a(it):
                    if it < 2:
                        xTi = xti_pre[it]
                    else:
                        xTi = big.tile([P, NDT, P], sdt, tag="xTi", bufs=2,
                                       name=f"xTi_{it}")
                        nc.sync.dma_start(
                            xTi[:], xT_views[it // 4][:, :, (it % 4) * P:(it % 4 + 1) * P]
                        )
                    # Two independent 1024-wide PSUM halves: half 0's row-max
                    # reduce hides under half 1's matmuls, and each half's
                    # buffer frees as soon as its own exp pass has read it.
                    # Within a half, ft outer / jc inner amortizes the
                    # stationary load across chunks.
                    sps2 = [
                        ps_s.tile([P, 1024], F32, tag=f"sps{h}", name=f"sps{h}_{it}")
                        for h in range(2)
                    ]
                    mx2 = work.tile([P, 2], F32, tag="mx2", bufs=2)
                    for half in range(2):
                        for ft in range(NDT):
                            for jcl in range(2):
                                nc.tensor.matmul(
                                    sps2[half][:, jcl * 512:(jcl + 1) * 512],
                                    xTi[:, ft, :],
                                    projF[:, ft, (half * 2 + jcl) * 512:(half * 2 + jcl + 1) * 512],
                                    start=(ft == 0),
                                    stop=(ft == NDT - 1),
                                )
                        nc.vector.reduce_max(
                            mx2[:, half:half + 1], sps2[half][:],
                            axis=mybir.AxisListType.X,
                        )
                    negm = work.tile([P, 1], F32, tag="negm", bufs=2)
                    nc.vector.reduce_max(negm[:], mx2[:], axis=mybir.AxisListType.X, negate=True)
                    psb = work.tile([P, S], F32, tag="psb", bufs=3)
                    lsum2 = work.tile([P, 2], F32, tag="lsum2", bufs=2)
                    for half in range(2):
                        nc.scalar.activation(
                            psb[:, half * 1024:(half + 1) * 1024], sps2[half][:],
                            mybir.ActivationFunctionType.Exp,
                            bias=negm[:], accum_out=lsum2[:, half:half + 1],
                        )
                    lsum = work.tile([P, 1], F32, tag="lsum", bufs=2)
                    nc.vector.reduce_sum(lsum[:], lsum2[:], axis=mybir.AxisListType.X)
                    rcp = work.tile([P, 1], F32, tag="rcp", bufs=2)
                    nc.vector.reciprocal(rcp[:], lsum[:])
                    nc.vector.tensor_scalar_mul(psb[:], psb[:], rcp[:])
                    nc.sync.dma_start(attn_ext[it * P:(it + 1) * P, :], psb[:])
                    a16 = work.tile([P, S], BF16, tag="a16", bufs=2)
                    nc.vector.tensor_copy(a16[:], psb[:])
                    a16_tiles[it] = a16

                def stage_b(it):
                    a16 = a16_tiles[it]
                    pt = work.tile([P, S], BF16, tag="pt", bufs=2)
                    for q in range(NIT // 8):
                        # 8 transposes per bank: a [128, 1024] bf16 tile is
                        # exactly one PSUM bank, halving the copy count.
                        trp = ps_b.tile([P, 1024], BF16, tag="trb")
                        for k in range(8):
                            jt = q * 8 + k
                            nc.tensor.transpose(
                                trp[:, k * P:(k + 1) * P],
                                a16[:, jt * P:(jt + 1) * P],
                                ident16[:],
                            )
                        nc.scalar.copy(pt[:, q * 1024:(q + 1) * 1024], trp[:])
                    cps = ps_b.tile([P, D], F32, tag="cps", bufs=1)
                    # jt outer / ec inner: stationary pt block reused across
                    # both 512-wide output chunks.
                    for jt in range(NIT):
                        for ec in range(NEC):
                            nc.tensor.matmul(
                                cps[:, ec * 512:(ec + 1) * 512],
                                pt[:, jt * P:(jt + 1) * P],
                                xbf[:, jt, ec * 512:(ec + 1) * 512],
                                start=(jt == 0),
                                stop=(jt == NIT - 1),
                            )
                    csb = work.tile([P, D], F32, tag="csb", bufs=2)
                    nc.scalar.copy(csb[:], cps[:])
                    nc.sync.dma_start(ctx_ext[it * P:(it + 1) * P, :], csb[:])

                for it in range(NIT):
                    stage_a(it)
                    if it >= 1:
                        stage_b(it - 1)
                stage_b(NIT - 1)


_compiled_nc = None


def _get_nc():
    global _compiled_nc
    if _compiled_nc is None:
        nc = bacc.Bacc("TRN2", target_bir_lowering=False, debug=False)
        _compiled_nc = build(nc)
    return _compiled_nc


LAST_EXEC_NS = None
LAST_RESULTS = None


def kernel(lstm_output, W, b):
    global LAST_EXEC_NS, LAST_RESULTS
    nc = _get_nc()
    x = np.ascontiguousarray(np.asarray(lstm_output, dtype=np.float32))
    w = np.ascontiguousarray(np.asarray(W, dtype=np.float32))
    bias = np.ascontiguousarray(np.asarray(b, dtype=np.float32))
    in_maps = [
        {"x": x[c], "w": w, "bias": bias} for c in range(N_CORES)
    ]
    trace = bool(int(os.environ.get("KERNEL_TRACE", "0")))
    res = run_bass_kernel_spmd(nc, in_maps, core_ids=list(range(N_CORES)), trace=trace)
    LAST_EXEC_NS = res.exec_time_ns
    LAST_RESULTS = res
    ctx = np.stack([res.results[c]["ctx"] for c in range(N_CORES)])
    attn = np.stack([res.results[c]["attn"] for c in range(N_CORES)])
    return ctx, attn
